# revision 36
# baseline (speedup 1.0000x reference)
"""Trainium2 Bass kernel for nn_CrossAttention (single-query cross attention).

Reference computation (B=4, C=64, H=W=128, heads h=64, dim_head d=64,
inner=4096, HW=16384):
    x[b, j, c]   = fimg[b, c, j]                       (j indexes H*W)
    q[b, h, d]   = sum_e fpsf[b, e] Wq[h*64+d, e]
    k[b, j, h, d]= sum_c x[b, j, c] Wk[h*64+d, c]
    out[b, h, j] = scale * sum_d q[b,h,d] k[b,j,h,d]

Single query per (batch, head) -> attention collapses:
    W2[b, h, c]  = scale * sum_d q[b,h,d] Wk[h*64+d, c]      (tiny)
    out[b, h, j] = sum_c W2[b,h,c] fimg[b, c, j]

Sharding: j (H*W = 16384) split across 8 cores (2048 each); every core
redundantly computes W2.

The kernel is HBM/DMA-latency bound: per nc.sync dma_start costs ~625ns
of serialized HWDGE descriptor generation + 650ns DGE->DMA delay + 900ns
semaphore propagation, and total traffic/core is what sets the body time.
This version minimizes DMA count + bytes:
  - all weights packed compactly (no block-diag zero padding in HBM):
    the block-diagonal needed for head-pair packing is placed in the tiny
    rhs (q2bd [128,256]) instead of the 1MB lhs.
  - output staged and stored as bf16 (halves output traffic; rel err
    stays ~4e-3 vs the 2e-2 gate), upcast to f32 on host.
  - A/B stages use 128-partition packed matmuls: 16+16 instrs vs 64.
  - DMAs all issued on the in-order SP queue in priority order
    (weights -> fimg chunks -> out chunks) so transfers stream
    back-to-back at full HBM rate with no issue-queue stalls on the
    critical path.

Device layouts (host does LAYOUT/dtype prep only, no math):
  wq_pack [128, 2056] bf16: cols 0:8 = fpsf block-diag (fb[e+64s, 4s+b] =
          fpsf[b,e]); cols 8:2056 = Wq packed:
          wq[e+64sg, 64p+d] = Wq[64*(2p+sg)+d, e]
  wk_pack [128, 2048] bf16: wk[d+64par, 128m+64r+c] = Wk[64*(4m+2r+par)+d, c]
  fimg_s  [128, 4096] bf16: rows 64*half+c, cols 2048*qp + j_local,
          batch b = 2*qp+half
  out     [128, 4096] bf16: rows 64*half+h, cols 2048*qp + j_local

Device compute per core (scale folded into fpsf_bd by one tiny vector op):
  A: 32 matmuls  q2a[d, 8p+4s+b] = scale*q[b, 2p+s, d]     (psum [64,256])
  q2bd copies (plain, block-diag in s): q2bd[d+64s, cols with s-field s]
     = q2a[d, same col]; 8 copies of [64, 32] (step-8)
  B: 16 matmuls  w2q[64r+c, 16m+8r+4s+b] = W2[b, 4m+2r+s, c]
     (lhsT = wk_pack cols 128m..; off-diagonal r-blocks are junk, ignored)
  Assembly: bd_qp[64half+c, 64half+(4m+2r+par)] = W2[2qp+half, 4m+2r+par, c]
  Big: 8 matmuls [128, 512] = bd_qp.T @ fimg chunk; psum -> bf16 staging
     [128, 1024] x4 (vector/scalar alternate); 4 output DMAs [128, 1024].
"""

import sys
import types

import numpy as np
import ml_dtypes

# antenv.axon_hooks is absent in this image; bass_utils imports it when
# tracing. Register a minimal stand-in before importing concourse.
if "antenv.axon_hooks" not in sys.modules:
    try:
        import antenv  # noqa: F401

        _hooks = types.ModuleType("antenv.axon_hooks")
        _hooks._hook = None

        def _set_hook(h):
            _hooks._hook = h

        _hooks.set_axon_ntff_profile_hook = _set_hook
        _hooks.get_axon_ntff_profile_hook = lambda: _hooks._hook
        sys.modules["antenv.axon_hooks"] = _hooks
        try:
            from trn_agent_boot.trn_boot import _ntff_profile_via_ctypes

            _set_hook(_ntff_profile_via_ctypes("/opt/axon/libaxon_pjrt.so"))
        except Exception:
            pass
    except ImportError:
        pass

import concourse.bass as bass  # noqa: E402
import concourse.mybir as mybir  # noqa: E402
import concourse.tile as tile  # noqa: E402
from concourse import bacc  # noqa: E402
from concourse.bass_utils import run_bass_kernel_spmd  # noqa: E402

N_CORES = 8
B, C, H, W = 4, 64, 128, 128
HEADS, DIM_HEAD = 64, 64
HW = H * W
JS = HW // N_CORES  # 2048 j-positions per core
SCALE = DIM_HEAD ** -0.5
F32 = mybir.dt.float32
BF16 = mybir.dt.bfloat16
NPBF16 = ml_dtypes.bfloat16

_compiled = None  # cache (nc) across calls


def _build():
    nc = bacc.Bacc("TRN2", target_bir_lowering=False, debug=False,
                   num_devices=N_CORES)

    w_d = nc.dram_tensor("w_pack", [128, 4104], BF16, kind="ExternalInput")
    img_d = nc.dram_tensor("fimg_s", [128, 4096], BF16, kind="ExternalInput")
    out_d = nc.dram_tensor("out", [128, 4096], BF16, kind="ExternalOutput")

    with tile.TileContext(nc) as tc:
        with (
            tc.tile_pool(name="weights", bufs=1) as wpool,
            tc.tile_pool(name="img", bufs=1) as ipool,
            tc.tile_pool(name="small_ps", bufs=1, space="PSUM") as spsum,
            tc.tile_pool(name="big_ps", bufs=7, space="PSUM") as bpsum,
            tc.tile_pool(name="ostage", bufs=8) as opool,
        ):
            # --- input DMAs, in priority order on the in-order SP queue:
            # wq chunks (gate A), wk chunks (gate B), then fimg chunks.
            w = wpool.tile([128, 4104], BF16, tag="w")
            for lo, hi in ((0, 1544), (1544, 2056), (2056, 4104)):
                nc.sync.dma_start(w[:, lo:hi], w_d.ap()[:, lo:hi])
            img = ipool.tile([128, 4096], BF16, tag="img")
            for c4 in range(4):
                nc.sync.dma_start(img[:, 1024 * c4:1024 * (c4 + 1)],
                                  img_d.ap()[:, 1024 * c4:1024 * (c4 + 1)])

            # dependency-free scalar op at main start so the lazy
            # ACT_TABLE_LOAD (~1.3us) runs hidden under the input DMAs
            # instead of stalling the first real scalar copy.
            warm = wpool.tile([128, 8], BF16, tag="warm")
            nc.gpsimd.memset(warm[:], 0.0)
            nc.scalar.copy(warm[0:1, 0:4], warm[0:1, 4:8])

            # scale folded once into the tiny fpsf block-diag
            fpsf_sc = wpool.tile([128, 8], BF16, tag="fpsf_sc")
            nc.vector.tensor_scalar_mul(fpsf_sc[:], w[:, 0:8], SCALE)

            # --- A: q2a[d, 8p+4s+b] = scale*q[b, 2p+s, d] ---
            # q2a and w2q share one PSUM bank (cols 0:256 / 256:512)
            sps = spsum.tile([128, 512], F32, tag="sps")
            q2a = sps[0:64, 0:256]
            for p in range(32):
                nc.tensor.matmul(
                    q2a[:, 8 * p:8 * p + 8],
                    w[:, 8 + 64 * p:8 + 64 * p + 64],
                    fpsf_sc[:],
                    start=True, stop=True,
                )

            # --- q2bd (bf16, block-diag in s-rows) ---
            # q2bd[d+64s, 16m+8s+4u+b] = scale*q[b, 4m+2u+s, d]; zeros at
            # mismatched s. One copy per s-half: contiguous inner writes,
            # disjoint partition ranges -> no false WAW serialization.
            q2bd = wpool.tile([128, 256], BF16, tag="q2bd")
            nc.gpsimd.memset(q2bd[:], 0.0)
            for s in range(2):
                dst = (q2bd[64 * s:64 * s + 64, :]
                       .rearrange("p (m f) -> p m f", f=16)
                       [:, :, 8 * s:8 * s + 8]
                       .rearrange("p m (u b) -> p m u b", u=2))
                src = (q2a[:, :]
                       .rearrange("p (m u x) -> p m u x", m=16, u=2, x=8)
                       [:, :, :, 4 * s:4 * s + 4])
                nc.vector.tensor_copy(dst, src)

            # --- B: w2q[64r+c, 16m+8s+4r+b] = W2[b, 4m+2r+s, c] ---
            # (valid at u-field == r; off-diagonal r-blocks are junk)
            w2q = sps[:, 256:512]
            for m in range(16):
                nc.tensor.matmul(
                    w2q[:, 16 * m:16 * m + 16],
                    w[:, 2056 + 128 * m:2056 + 128 * m + 128],
                    q2bd[:, 16 * m:16 * m + 16],
                    start=True, stop=True,
                )

            # --- assembly into one fused tile bd01 [128, 256]:
            #     bd01[64half+c, 128qp + 64half+32r+16s+m] =
            #     W2[2qp+half, 4m+2r+s, c]  (col relabeled; the host
            #     unpack inverts the (r,s,m)->h permutation for free).
            #     4 copies with contiguous disjoint writes; big-mm lhsT
            #     slices bd01 per qp.
            bd01 = wpool.tile([128, 256], BF16, tag="bd01")
            nc.gpsimd.memset(bd01[:], 0.0)
            for half in range(2):
                for r in range(2):
                    dst = (bd01[64 * half:64 * half + 64, :]
                           .rearrange("p (qp x) -> p qp x", qp=2)
                           [:, :, 64 * half + 32 * r:64 * half + 32 * r + 32]
                           .rearrange("p qp (s m) -> p qp s m", s=2))
                    src = (w2q[64 * r:64 * r + 64, :]
                           .rearrange("p (m s x2) -> p x2 s m", m=16, s=2)
                           [:, 4 * r + half:4 * r + half + 3:2, :, :])
                    nc.vector.tensor_copy(dst, src)

            # --- big matmuls + bf16 staging + output DMAs ---
            # 8 chunks of 512; staging alternates vector/scalar; the out
            # DMA issues alternate SP/Act so neither queue head-of-line
            # blocks the staging pipeline.
            for k in range(8):
                qp = k // 4
                ps = bpsum.tile([128, 512], F32, tag="mm_ps")
                nc.tensor.matmul(
                    ps[:], bd01[:, 128 * qp:128 * qp + 128],
                    img[:, 512 * k:512 * k + 512],
                    start=True, stop=True,
                )
                ot = opool.tile([128, 512], BF16, tag="ot")
                if k % 2 == 0:
                    nc.vector.tensor_copy(ot[:], ps[:])
                    nc.sync.dma_start(
                        out_d.ap()[:, 512 * k:512 * k + 512], ot[:])
                else:
                    nc.scalar.copy(ot[:], ps[:])
                    nc.scalar.dma_start(
                        out_d.ap()[:, 512 * k:512 * k + 512], ot[:])

    nc.compile()
    return nc


def _prep_inputs(fpsf, fimg, Wq, Wk):
    fpsf = np.ascontiguousarray(fpsf, dtype=np.float32)
    fimg = np.ascontiguousarray(fimg, dtype=np.float32)
    Wq = np.ascontiguousarray(Wq, dtype=np.float32)
    Wk = np.ascontiguousarray(Wk, dtype=np.float32)

    wp = np.zeros((128, 4104), dtype=NPBF16)
    fb = np.zeros((2, 64, 2, 4), np.float32)
    fb[0, :, 0, :] = fpsf.T
    fb[1, :, 1, :] = fpsf.T
    wp[:, 0:8] = fb.reshape(128, 8).astype(NPBF16)
    # wq[e+64sg, 64p+d] = Wq[64*(2p+sg)+d, e]
    wp[:, 8:2056] = (
        Wq.reshape(32, 2, 64, 64).transpose(1, 3, 0, 2)
        .reshape(128, 2048).astype(NPBF16)
    )
    # wk[d+64par, 128m+64r+c] = Wk[64*(4m+2r+par)+d, c]
    wp[:, 2056:4104] = (
        Wk.reshape(16, 2, 2, 64, 64).transpose(2, 3, 0, 1, 4)
        .reshape(128, 2048).astype(NPBF16)
    )

    fimg16 = fimg.reshape(B, C, HW).astype(NPBF16)
    in_maps = []
    for i in range(N_CORES):
        x = fimg16[:, :, JS * i:JS * (i + 1)]  # [4, 64, 2048]
        sh = np.ascontiguousarray(
            x.reshape(2, 2, 64, JS).transpose(1, 2, 0, 3).reshape(128, 2 * JS)
        )
        in_maps.append({
            "fimg_s": sh,
            "w_pack": wp,
        })
    return in_maps


def kernel(fpsf, fimg, Wq, Wk):
    global _compiled
    if _compiled is None:
        _compiled = _build()
    nc = _compiled

    in_maps = _prep_inputs(fpsf, fimg, Wq, Wk)
    res = run_bass_kernel_spmd(nc, in_maps, core_ids=list(range(N_CORES)))

    out = np.empty((B, HEADS, HW), dtype=np.float32)
    for i in range(N_CORES):
        # res [128, 4096] bf16: rows (half, r, s, m) = 64half+32r+16s+m,
        # cols (qp, j); b = 2qp+half, h = 4m+2r+s
        a = res.results[i]["out"].reshape(2, 2, 2, 16, 2, JS)
        out[:, :, JS * i:JS * (i + 1)] = (
            a.transpose(4, 0, 3, 1, 2, 5).reshape(4, 64, JS)
            .astype(np.float32)
        )
    return out.reshape(B, C, H, W)


if __name__ == "__main__":
    rng = np.random.default_rng(0)
    ins = {
        "fpsf": rng.standard_normal((B, C), dtype=np.float32),
        "fimg": rng.standard_normal((B, C, H, W), dtype=np.float32),
        "Wq": (rng.standard_normal((4096, C), dtype=np.float32) * 0.05),
        "Wk": (rng.standard_normal((4096, C), dtype=np.float32) * 0.05),
    }
    out = kernel(**ins)
    print("out", out.shape, out.dtype, float(np.abs(out).max()))


# revision 38
# speedup vs baseline: 1.0067x; 1.0067x over previous
"""Trainium2 Bass kernel for nn_CrossAttention (single-query cross attention).

Reference computation (B=4, C=64, H=W=128, heads h=64, dim_head d=64,
inner=4096, HW=16384):
    x[b, j, c]   = fimg[b, c, j]                       (j indexes H*W)
    q[b, h, d]   = sum_e fpsf[b, e] Wq[h*64+d, e]
    k[b, j, h, d]= sum_c x[b, j, c] Wk[h*64+d, c]
    out[b, h, j] = scale * sum_d q[b,h,d] k[b,j,h,d]

Single query per (batch, head) -> attention collapses:
    W2[b, h, c]  = scale * sum_d q[b,h,d] Wk[h*64+d, c]      (tiny)
    out[b, h, j] = sum_c W2[b,h,c] fimg[b, c, j]

Sharding: j (H*W = 16384) split across 8 cores (2048 each); every core
redundantly computes W2.

Measured TRN2 cost structure this kernel is built around:
  - entry (cross-core barrier + engine program load) ~7us and exit drain
    ~3-4us are fixed NEFF overhead; a trivial copy kernel measures ~15.6us.
  - each dma_start costs ~0.62us of HWDGE descriptor-generation on the
    issuing queue (SP and Act queues generate in parallel), ~0.65us
    DGE->DMA delay, and +0.9us semaphore propagation after the transfer.
  - the 16 DMA rings sustain ~360GB/s; the 2.08MB input stream is the
    single largest body cost (~5.5-6us).
  - vector/scalar ops that touch a tile through a strided or multi-dim
    view get conservative whole-tile dependency treatment by the Tile
    scheduler and serialize in program order (~0.1us gaps); only plain
    contiguous slices track precise ranges and overlap across engines.

Optimizations vs the naive version (29.4us -> ~25.5us):
  - bf16 output staging + store (halves output bytes; host upcasts).
  - compact weights, no zero padding in HBM: the block-diagonal needed
    for head-pair packing lives in the tiny rhs q2bd [128,256] (A) and
    in junk-block outputs that are simply never read (B).
  - one packed weight tensor, DMA'd in 3 chunks on the in-order SP queue
    in priority order (wq gates A, wk gates B) ahead of fimg, so the
    critical W2 chain streams while fimg transfers.
  - ACT_TABLE_LOAD (1.3us, lazy) pulled to kernel start by a dummy
    dependency-free scalar op.
  - all strided repack copies use as few instructions as possible (2 for
    q2bd, 4 for bd01) since they serialize; plain big staging copies
    alternate vector/scalar and do overlap.
  - output in 8 chunks; DMA issues alternate SP/Act (parallel HWDGE
    generation) so the staging pipeline is never head-of-line blocked.

Device layouts (host does LAYOUT/dtype prep only, no math):
  w_pack [128, 4104] bf16: cols 0:8 fpsf block-diag fb[e+64s, 4s+b] =
         fpsf[b,e]; cols 8:2056 wq[e+64sg, 64p+d] = Wq[64*(2p+sg)+d, e];
         cols 2056:4104 wk[d+64par, 128m+64r+c] = Wk[64*(4m+2r+par)+d, c]
  fimg_s [128, 4096] bf16: rows 64*half+c, cols 2048*qp + j_local,
         batch b = 2*qp+half
  out    [128, 4096] bf16: rows 64half+32r+16s+m (head h = 4m+2r+s),
         cols 2048*qp + j_local (host unpack inverts the permutation)

Device compute per core (scale folded into fpsf_bd by one tiny vector op):
  A: 32 matmuls  q2a[d, 8p+4s+b] = scale*q[b, 2p+s, d]     (psum [64,256])
  q2bd: 2 copies, block-diag in s-rows: q2bd[d+64s, 16m+8s+4u+b]
     = scale*q[b, 4m+2u+s, d], zeros at mismatched s
  B: 16 matmuls  w2q[64r+c, 16m+8s+4r+b] = W2[b, 4m+2r+s, c]
     (valid where the rhs u-field == r; junk blocks never read)
  Assembly: 4 copies into bd01[64half+c, 128qp+64half+32r+16s+m]
  Big: 8 matmuls [128, 512] = bd01 slice @ fimg chunk; psum -> bf16
     staging [128, 512] (vector/scalar alternate); 8 output DMAs.
"""

import sys
import types

import numpy as np
import ml_dtypes

# antenv.axon_hooks is absent in this image; bass_utils imports it when
# tracing. Register a minimal stand-in before importing concourse.
if "antenv.axon_hooks" not in sys.modules:
    try:
        import antenv  # noqa: F401

        _hooks = types.ModuleType("antenv.axon_hooks")
        _hooks._hook = None

        def _set_hook(h):
            _hooks._hook = h

        _hooks.set_axon_ntff_profile_hook = _set_hook
        _hooks.get_axon_ntff_profile_hook = lambda: _hooks._hook
        sys.modules["antenv.axon_hooks"] = _hooks
        try:
            from trn_agent_boot.trn_boot import _ntff_profile_via_ctypes

            _set_hook(_ntff_profile_via_ctypes("/opt/axon/libaxon_pjrt.so"))
        except Exception:
            pass
    except ImportError:
        pass

import concourse.bass as bass  # noqa: E402
import concourse.mybir as mybir  # noqa: E402
import concourse.tile as tile  # noqa: E402
from concourse import bacc  # noqa: E402
from concourse.bass_utils import run_bass_kernel_spmd  # noqa: E402

N_CORES = 8
B, C, H, W = 4, 64, 128, 128
HEADS, DIM_HEAD = 64, 64
HW = H * W
JS = HW // N_CORES  # 2048 j-positions per core
SCALE = DIM_HEAD ** -0.5
F32 = mybir.dt.float32
BF16 = mybir.dt.bfloat16
NPBF16 = ml_dtypes.bfloat16

_compiled = None  # cache (nc) across calls


def _build():
    nc = bacc.Bacc("TRN2", target_bir_lowering=False, debug=False,
                   num_devices=N_CORES)

    w_d = nc.dram_tensor("w_pack", [128, 4104], BF16, kind="ExternalInput")
    img_d = nc.dram_tensor("fimg_s", [128, 4096], BF16, kind="ExternalInput")
    out_d = nc.dram_tensor("out", [128, 4096], BF16, kind="ExternalOutput")

    with tile.TileContext(nc) as tc:
        with (
            tc.tile_pool(name="weights", bufs=1) as wpool,
            tc.tile_pool(name="img", bufs=1) as ipool,
            tc.tile_pool(name="small_ps", bufs=1, space="PSUM") as spsum,
            tc.tile_pool(name="big_ps", bufs=7, space="PSUM") as bpsum,
            tc.tile_pool(name="ostage", bufs=8) as opool,
        ):
            # --- input DMAs, in priority order on the in-order SP queue:
            # wq chunks (gate A), wk chunks (gate B), then fimg chunks.
            w = wpool.tile([128, 4104], BF16, tag="w")
            for lo, hi in ((0, 1032), (1032, 2056), (2056, 4104)):
                nc.sync.dma_start(w[:, lo:hi], w_d.ap()[:, lo:hi])
            img = ipool.tile([128, 4096], BF16, tag="img")
            for c4 in range(4):
                nc.sync.dma_start(img[:, 1024 * c4:1024 * (c4 + 1)],
                                  img_d.ap()[:, 1024 * c4:1024 * (c4 + 1)])

            # dependency-free scalar op at main start so the lazy
            # ACT_TABLE_LOAD (~1.3us) runs hidden under the input DMAs
            # instead of stalling the first real scalar copy.
            warm = wpool.tile([128, 8], BF16, tag="warm")
            nc.gpsimd.memset(warm[:], 0.0)
            nc.scalar.copy(warm[0:1, 0:4], warm[0:1, 4:8])

            # scale folded once into the tiny fpsf block-diag
            fpsf_sc = wpool.tile([128, 8], BF16, tag="fpsf_sc")
            nc.vector.tensor_scalar_mul(fpsf_sc[:], w[:, 0:8], SCALE)

            # --- A: q2a[d, 8p+4s+b] = scale*q[b, 2p+s, d] ---
            # q2a and w2q share one PSUM bank (cols 0:256 / 256:512)
            sps = spsum.tile([128, 512], F32, tag="sps")
            q2a = sps[0:64, 0:256]
            for p in range(32):
                nc.tensor.matmul(
                    q2a[:, 8 * p:8 * p + 8],
                    w[:, 8 + 64 * p:8 + 64 * p + 64],
                    fpsf_sc[:],
                    start=True, stop=True,
                )

            # --- q2bd (bf16, block-diag in s-rows) ---
            # q2bd[d+64s, 16m+8s+4u+b] = scale*q[b, 4m+2u+s, d]; zeros at
            # mismatched s. One copy per s-half: contiguous inner writes,
            # disjoint partition ranges -> no false WAW serialization.
            q2bd = wpool.tile([128, 256], BF16, tag="q2bd")
            nc.gpsimd.memset(q2bd[:], 0.0)
            for s in range(2):
                dst = (q2bd[64 * s:64 * s + 64, :]
                       .rearrange("p (m f) -> p m f", f=16)
                       [:, :, 8 * s:8 * s + 8]
                       .rearrange("p m (u b) -> p m u b", u=2))
                src = (q2a[:, :]
                       .rearrange("p (m u x) -> p m u x", m=16, u=2, x=8)
                       [:, :, :, 4 * s:4 * s + 4])
                nc.vector.tensor_copy(dst, src)

            # --- B: w2q[64r+c, 16m+8s+4r+b] = W2[b, 4m+2r+s, c] ---
            # (valid at u-field == r; off-diagonal r-blocks are junk)
            w2q = sps[:, 256:512]
            for m in range(16):
                nc.tensor.matmul(
                    w2q[:, 16 * m:16 * m + 16],
                    w[:, 2056 + 128 * m:2056 + 128 * m + 128],
                    q2bd[:, 16 * m:16 * m + 16],
                    start=True, stop=True,
                )

            # --- assembly into one fused tile bd01 [128, 256]:
            #     bd01[64half+c, 128qp + 64half+32r+16s+m] =
            #     W2[2qp+half, 4m+2r+s, c]  (col relabeled; the host
            #     unpack inverts the (r,s,m)->h permutation for free).
            #     4 copies with contiguous disjoint writes; big-mm lhsT
            #     slices bd01 per qp.
            bd01 = wpool.tile([128, 256], BF16, tag="bd01")
            nc.gpsimd.memset(bd01[:], 0.0)
            for half in range(2):
                for r in range(2):
                    dst = (bd01[64 * half:64 * half + 64, :]
                           .rearrange("p (qp x) -> p qp x", qp=2)
                           [:, :, 64 * half + 32 * r:64 * half + 32 * r + 32]
                           .rearrange("p qp (s m) -> p qp s m", s=2))
                    src = (w2q[64 * r:64 * r + 64, :]
                           .rearrange("p (m s x2) -> p x2 s m", m=16, s=2)
                           [:, 4 * r + half:4 * r + half + 3:2, :, :])
                    nc.vector.tensor_copy(dst, src)

            # --- big matmuls + bf16 staging + output DMAs ---
            # 8 chunks of 512; staging alternates vector/scalar; the out
            # DMA issues alternate SP/Act so neither queue head-of-line
            # blocks the staging pipeline.
            for k in range(8):
                qp = k // 4
                ps = bpsum.tile([128, 512], F32, tag="mm_ps")
                nc.tensor.matmul(
                    ps[:], bd01[:, 128 * qp:128 * qp + 128],
                    img[:, 512 * k:512 * k + 512],
                    start=True, stop=True,
                )
                ot = opool.tile([128, 512], BF16, tag="ot")
                if k % 2 == 0:
                    nc.vector.tensor_copy(ot[:], ps[:])
                    nc.sync.dma_start(
                        out_d.ap()[:, 512 * k:512 * k + 512], ot[:])
                else:
                    nc.scalar.copy(ot[:], ps[:])
                    nc.scalar.dma_start(
                        out_d.ap()[:, 512 * k:512 * k + 512], ot[:])

    nc.compile()
    return nc


def _prep_inputs(fpsf, fimg, Wq, Wk):
    fpsf = np.ascontiguousarray(fpsf, dtype=np.float32)
    fimg = np.ascontiguousarray(fimg, dtype=np.float32)
    Wq = np.ascontiguousarray(Wq, dtype=np.float32)
    Wk = np.ascontiguousarray(Wk, dtype=np.float32)

    wp = np.zeros((128, 4104), dtype=NPBF16)
    fb = np.zeros((2, 64, 2, 4), np.float32)
    fb[0, :, 0, :] = fpsf.T
    fb[1, :, 1, :] = fpsf.T
    wp[:, 0:8] = fb.reshape(128, 8).astype(NPBF16)
    # wq[e+64sg, 64p+d] = Wq[64*(2p+sg)+d, e]
    wp[:, 8:2056] = (
        Wq.reshape(32, 2, 64, 64).transpose(1, 3, 0, 2)
        .reshape(128, 2048).astype(NPBF16)
    )
    # wk[d+64par, 128m+64r+c] = Wk[64*(4m+2r+par)+d, c]
    wp[:, 2056:4104] = (
        Wk.reshape(16, 2, 2, 64, 64).transpose(2, 3, 0, 1, 4)
        .reshape(128, 2048).astype(NPBF16)
    )

    fimg16 = fimg.reshape(B, C, HW).astype(NPBF16)
    in_maps = []
    for i in range(N_CORES):
        x = fimg16[:, :, JS * i:JS * (i + 1)]  # [4, 64, 2048]
        sh = np.ascontiguousarray(
            x.reshape(2, 2, 64, JS).transpose(1, 2, 0, 3).reshape(128, 2 * JS)
        )
        in_maps.append({
            "fimg_s": sh,
            "w_pack": wp,
        })
    return in_maps


def kernel(fpsf, fimg, Wq, Wk):
    global _compiled
    if _compiled is None:
        _compiled = _build()
    nc = _compiled

    in_maps = _prep_inputs(fpsf, fimg, Wq, Wk)
    res = run_bass_kernel_spmd(nc, in_maps, core_ids=list(range(N_CORES)))

    out = np.empty((B, HEADS, HW), dtype=np.float32)
    for i in range(N_CORES):
        # res [128, 4096] bf16: rows (half, r, s, m) = 64half+32r+16s+m,
        # cols (qp, j); b = 2qp+half, h = 4m+2r+s
        a = res.results[i]["out"].reshape(2, 2, 2, 16, 2, JS)
        out[:, :, JS * i:JS * (i + 1)] = (
            a.transpose(4, 0, 3, 1, 2, 5).reshape(4, 64, JS)
            .astype(np.float32)
        )
    return out.reshape(B, C, H, W)


if __name__ == "__main__":
    rng = np.random.default_rng(0)
    ins = {
        "fpsf": rng.standard_normal((B, C), dtype=np.float32),
        "fimg": rng.standard_normal((B, C, H, W), dtype=np.float32),
        "Wq": (rng.standard_normal((4096, C), dtype=np.float32) * 0.05),
        "Wk": (rng.standard_normal((4096, C), dtype=np.float32) * 0.05),
    }
    out = kernel(**ins)
    print("out", out.shape, out.dtype, float(np.abs(out).max()))
